# revision 8
# baseline (speedup 1.0000x reference)
"""Viterbi CRF decode kernel for Trainium2 (8 NeuronCores, data-parallel).

Device computes the forward DP (max-plus scan) per sequence and streams the
full fv history back; the host recomputes backpointers only along the taken
path (exact in f32) and does the backtrace.

Layout per core: 256 sequences = 2 tiles x 128 partitions; tags in the free
dim. Per step: cand[s,n,p] = fv[s,p] + trans[n,p] (tensor_tensor with a
broadcast AP), grouped reduce-max over p, then + feat[s,t,n].

Sequences are length-sorted on the host into 16 tiles; core c gets tiles
(c, 15-c) so every core sees ~the same total work. Each tile's time loop is
split into 64-step chunks; chunks past ceil(maxlen/64) are skipped at runtime
via nested tile-If blocks on a register loaded from the per-core "nch" input.
"""
import sys

sys.path.insert(0, "/opt/trn_rl_repo")

import numpy as np
from contextlib import ExitStack

import concourse.bass as bass
import concourse.bacc as bacc
import concourse.tile as tile
import concourse.mybir as mybir
from concourse.bass_utils import run_bass_kernel_spmd

# ---------------------------------------------------------------------------
# Custom DVE op: segmented max-plus (SEG_MAXPLUS_ANT).
# out[p,s,k] = running max over k'<=k of (in0[p,s,k'] + in1[p,s,k']), with the
# running max reset at every page boundary s. The per-page max lands at
# out[p,s,N-1]. Built from Spec(body=scan(MAX, Src0+Src1)) -> [seed, steady],
# plus a hand-constructed SUB_DIM_DONE step state that re-seeds the scan stage
# for exactly the first element of each new page:
#     steady: d <- MAX(CURR_ALU_OUT, expr)   (running max)
#     step:   d <- MAX(MaxNeg, expr) = expr  (reset + absorb first element)
# mirroring the PageIdx 3-uop machine's FSM wiring. HW-validated exact.
# ---------------------------------------------------------------------------
import dataclasses as _dc
import concourse.dve_spec as _DS
from concourse.dve_uop import N_LANES as _N_LANES, N_STAGES as _N_STAGES
from concourse.dve_uop import DveOpSpec as _DveOpSpec
import concourse.dve_ops as _DO


def _segmaxplus_reference(in0, in1, c0, c1, c2):
    s = np.asarray(in0, np.float32) + np.asarray(in1, np.float32)
    return np.maximum.accumulate(s, axis=-1)


def _segmaxplus_build(ver):
    n_lanes, n_stages = _N_LANES[ver], _N_STAGES[ver]
    spec = _DS.Spec(
        body=_DS.scan(_DS.AluOp.MAX, _DS.Src0 + _DS.Src1),
        reference=_segmaxplus_reference,
    )
    spec2 = _DS._hoist_stream_invariant_ops(spec)
    scans = _DS._collect(spec2.body, _DS.Scan)
    latches = _DS._collect(spec2.body, _DS.Latch)
    placement = _DS._build_placement(spec2, scans, n_stages, n_lanes)
    states = _DS._build_state_machine(spec2, scans, latches, placement)
    assert len(states) == 2  # [seed, steady]
    scan_node = scans[0]
    d = placement.node_stage[scan_node]
    steady2 = _dc.replace(
        states[1],
        trigger=(_DS.Trigger.SRC_TENSOR_DONE, _DS.Trigger.SUB_DIM_DONE,
                 _DS.Trigger.NONE),
        next=(0, 2, 0),
    )
    step = _DS._State(
        placement=placement,
        consume=states[1].consume,
        overrides={d: _DS._Stage(_DS.AluOp.MAX, _DS.MaxNeg, scan_node.expr)},
        trigger=(_DS.Trigger.SRC_TENSOR_DONE, _DS.Trigger.SUB_DIM_DONE,
                 _DS.Trigger.COUNT),
        next=(0, 2, 1),
        repeat=1,
    )
    uops = [_DS._assemble(s) for s in (states[0], steady2, step)]
    for u in uops:
        u.validate(ver)
    return spec, uops


class _SegMaxPlusOp:
    name = "SEG_MAXPLUS_ANT"
    subdim = True

    def __init__(self):
        self.spec, _ = _segmaxplus_build("v3")
        self._compiled = {}

    def compile(self, ver):
        if ver not in self._compiled:
            _, uops = _segmaxplus_build(ver)
            self._compiled[ver] = _DveOpSpec(
                name=self.name,
                opcode=_DO.get_dve_sub_opcode(self.name),
                uops=uops,
                rd1_en=True,
            )
        return self._compiled[ver]


def _register_segmaxplus():
    for o in _DO.OPS:
        if o.name == _SegMaxPlusOp.name:
            return o
    op = _SegMaxPlusOp()
    _DO.OPS.append(op)
    _DO._SUB_OPCODE_FOR_NAME[op.name] = max(_DO._SUB_OPCODE_FOR_NAME.values()) + 1
    _DO.CUSTOM_DVE_SPECS[op.name] = op.spec
    return op


SEG_MAXPLUS = _register_segmaxplus()

B, L, T = 2048, 512, 24
NCORES = 8
SEQ_PER_CORE = B // NCORES  # 256
TILES = 2  # tiles of 128 partitions per core
TP = 128
CHUNK = 64  # time steps per If/DMA chunk
NCH = L // CHUNK  # 8
START_ID, STOP_ID, PAD_ID = 24, 25, 23

F32 = mybir.dt.float32
I32 = mybir.dt.int32
_cache = {}


def _build():
    if "nc" in _cache:
        return _cache["nc"]
    nc = bacc.Bacc("TRN2", target_bir_lowering=False, debug=False, num_devices=NCORES)

    feats_in = nc.dram_tensor("feats", [SEQ_PER_CORE, L, T], F32, kind="ExternalInput")
    init_in = nc.dram_tensor("init_fv", [SEQ_PER_CORE, T], F32, kind="ExternalInput")
    trans_in = nc.dram_tensor("trans_rep", [TP, T * T], F32, kind="ExternalInput")
    nch_in = nc.dram_tensor("nch", [1, TILES], I32, kind="ExternalInput")
    fvh_out = nc.dram_tensor("fvh", [SEQ_PER_CORE, L, T], F32, kind="ExternalOutput")

    with tile.TileContext(nc) as tc:
        with ExitStack() as ctx:
            const_pool = ctx.enter_context(tc.tile_pool(name="const", bufs=1))
            feat_pools = [
                ctx.enter_context(tc.tile_pool(name=f"feat{i}", bufs=1))
                for i in range(TILES)
            ]
            fvh_pools = [
                ctx.enter_context(tc.tile_pool(name=f"fvh{i}", bufs=1))
                for i in range(TILES)
            ]
            cand_pools = [
                ctx.enter_context(tc.tile_pool(name=f"cand{i}", bufs=2))
                for i in range(TILES)
            ]
            mx_pools = [
                ctx.enter_context(tc.tile_pool(name=f"mx{i}", bufs=2))
                for i in range(TILES)
            ]

            trans_sb = const_pool.tile([TP, T * T], F32, name="trans_sb")
            nc.sync.dma_start(trans_sb[:], trans_in.ap())
            trans_3d = trans_sb[:].rearrange("p (g k) -> p g k", g=T)

            nch_sb = const_pool.tile([1, TILES], I32, name="nch_sb")
            nc.sync.dma_start(nch_sb[:], nch_in.ap())
            regs = [
                nc.alloc_registers(
                    f"nch{i}",
                    bass.OrderedSet([mybir.EngineType.DVE, mybir.EngineType.SP]),
                )
                for i in range(TILES)
            ]
            sv = []
            for i in range(TILES):
                nc.regs_load(regs[i], nch_sb[0:1, i : i + 1])
                sv.append(nc.snap(regs[i], donate=False, min_val=1, max_val=NCH))

            # whole-tile feat prefetch (one big DMA per tile)
            feat_sb = []
            for ti in range(TILES):
                s0 = ti * TP
                fsb = feat_pools[ti].tile([TP, L * T], F32, name=f"featsb{ti}")
                nc.sync.dma_start(
                    fsb[:], feats_in[s0 : s0 + TP, :, :].rearrange("p a b -> p (a b)")
                )
                feat_sb.append(fsb)

            # static double-buffered fvh chunk tiles (pool allocation inside
            # If blocks deadlocks the tile scheduler)
            hbufs = [
                [
                    fvh_pools[ti].tile([TP, CHUNK * T], F32, name=f"hb{ti}_{j}")
                    for j in range(2)
                ]
                for ti in range(TILES)
            ]
            prev_fv = [None] * TILES

            def emit_chunk(ti, k):
                s0 = ti * TP
                fvh_ch = hbufs[ti][k % 2]
                if k == 0:
                    nc.sync.dma_start(fvh_ch[:, 0:T], init_in[s0 : s0 + TP, :])
                    prev_fv[ti] = fvh_ch[:, 0:T]
                for lt in range(CHUNK):
                    t = k * CHUNK + lt
                    if t == 0:
                        continue
                    seg = cand_pools[ti].tile(
                        [TP, T * T], F32, tag="c", name=f"seg{ti}"
                    )
                    seg_3d = seg[:].rearrange("p (g k) -> p g k", g=T)
                    fv_b = prev_fv[ti].unsqueeze(1).broadcast_to((TP, T, T))
                    # one fused op: seg[p,n,23] = max_p'(trans[n,p'] + fv[p'])
                    nc.vector._custom_dve(
                        SEG_MAXPLUS, out=seg_3d, in0=trans_3d, in1=fv_b
                    )
                    slot = fvh_ch[:, lt * T : (lt + 1) * T]
                    nc.vector.tensor_add(
                        slot, seg[:, T - 1 :: T], feat_sb[ti][:, t * T : (t + 1) * T]
                    )
                    prev_fv[ti] = slot
                nc.sync.dma_start(
                    fvh_out[s0 : s0 + TP, k * CHUNK : (k + 1) * CHUNK, :],
                    fvh_ch[:].rearrange("p (c t) -> p c t", c=CHUNK),
                )

            # chunk 0 unconditional (both tiles interleave); chunks 1..7 inside
            # nested Ifs, one chain per tile (If contexts must nest strictly)
            for ti in range(TILES):
                emit_chunk(ti, 0)
            for ti in range(TILES):
                with ExitStack() as stk:
                    for k in range(1, NCH):
                        stk.enter_context(tc.If(sv[ti] > k, name=f"t{ti}c{k}"))
                        emit_chunk(ti, k)

    nc.compile()
    _cache["nc"] = nc
    return nc


def _layout(lengths):
    """Device layout permutation: perm[i] = original seq index at device row i.
    Core c rows = [tile_c ; tile_{15-c}] of the length-sorted order."""
    order = np.argsort(lengths, kind="stable")
    tiles = [order[t * TP : (t + 1) * TP] for t in range(2 * NCORES)]
    perm, nch = [], []
    for c in range(NCORES):
        ta, tb = c, 2 * NCORES - 1 - c
        perm.append(tiles[ta])
        perm.append(tiles[tb])
        wa = max(1, -(-int(lengths[tiles[ta]].max()) // CHUNK))
        wb = max(1, -(-int(lengths[tiles[tb]].max()) // CHUNK))
        nch.append((wa, wb))
    return np.concatenate(perm), nch


def run_device(feats, init_fv, trans_rep, perm, nch, trace=False):
    nc = _build()
    feats_d = feats[perm]
    init_d = init_fv[perm]
    in_maps = []
    for c in range(NCORES):
        s0 = c * SEQ_PER_CORE
        in_maps.append(
            {
                "feats": np.ascontiguousarray(feats_d[s0 : s0 + SEQ_PER_CORE]),
                "init_fv": np.ascontiguousarray(init_d[s0 : s0 + SEQ_PER_CORE]),
                "trans_rep": trans_rep,
                "nch": np.array([nch[c]], dtype=np.int32),
            }
        )
    res = run_bass_kernel_spmd(nc, in_maps, list(range(NCORES)), trace=trace)
    fvh = np.concatenate([res.results[c]["fvh"] for c in range(NCORES)], axis=0)
    return fvh, res


def kernel(feats, feats_mask, transitions):
    feats = np.asarray(feats, dtype=np.float32)
    feats_mask = np.asarray(feats_mask)
    transitions = np.asarray(transitions, dtype=np.float32)

    lengths = feats_mask.astype(np.int64).sum(axis=1)  # prefix mask, >= 1
    trans24 = transitions[:T, :T]
    trans_rep = np.ascontiguousarray(
        np.broadcast_to(trans24.reshape(1, T * T), (TP, T * T)), dtype=np.float32
    )
    init_fv = (transitions[:T, START_ID][None, :] + feats[:, 0, :]).astype(np.float32)

    perm, nch = _layout(lengths)
    fvh, _ = run_device(feats, init_fv, trans_rep, perm, nch)
    len_d = lengths[perm]

    # ---- host backtrace in device order (exact f32, matches jax reference) ----
    idx = np.arange(B)
    fv_last = fvh[idx, len_d - 1, :]  # frozen fv at the last valid step
    terminal = fv_last + transitions[STOP_ID, :T][None, :]
    best = np.argmax(terminal, axis=1)
    scores_d = terminal[idx, best].astype(np.float32)

    paths_d = np.full((B, L), PAD_ID, dtype=np.int32)
    cur = best.astype(np.int64)
    for t in range(L - 1, 0, -1):
        valid = t < len_d  # mask[:, t] as bool
        paths_d[valid, t] = cur[valid]
        cand = fvh[:, t - 1, :] + trans24[cur]  # [B, T]
        prev = np.argmax(cand, axis=1)
        cur = np.where(valid, prev, cur)
    paths_d[:, 0] = cur

    paths = np.empty_like(paths_d)
    scores = np.empty_like(scores_d)
    paths[perm] = paths_d
    scores[perm] = scores_d
    return paths.astype(np.int32), scores


# revision 9
# speedup vs baseline: 1.0375x; 1.0375x over previous
"""Viterbi CRF decode kernel for Trainium2 (8 NeuronCores, data-parallel).

Device computes the forward DP (max-plus scan) per sequence and streams the
full fv history back; the host recomputes backpointers only along the taken
path (exact in f32) and does the backtrace.

Layout per core: 256 sequences = 2 tiles x 128 partitions; tags in the free
dim. Per step: cand[s,n,p] = fv[s,p] + trans[n,p] (tensor_tensor with a
broadcast AP), grouped reduce-max over p, then + feat[s,t,n].

Sequences are length-sorted on the host into 16 tiles; core c gets tiles
(c, 15-c) so every core sees ~the same total work. Each tile's time loop is
split into 64-step chunks; chunks past ceil(maxlen/64) are skipped at runtime
via nested tile-If blocks on a register loaded from the per-core "nch" input.
"""
import sys

sys.path.insert(0, "/opt/trn_rl_repo")

import numpy as np
from contextlib import ExitStack

import concourse.bass as bass
import concourse.bacc as bacc
import concourse.tile as tile
import concourse.mybir as mybir
from concourse.bass_utils import run_bass_kernel_spmd

# ---------------------------------------------------------------------------
# Custom DVE op: segmented max-plus (SEG_MAXPLUS_ANT).
# out[p,s,k] = running max over k'<=k of (in0[p,s,k'] + in1[p,s,k']), with the
# running max reset at every page boundary s. The per-page max lands at
# out[p,s,N-1]. Built from Spec(body=scan(MAX, Src0+Src1)) -> [seed, steady],
# plus a hand-constructed SUB_DIM_DONE step state that re-seeds the scan stage
# for exactly the first element of each new page:
#     steady: d <- MAX(CURR_ALU_OUT, expr)   (running max)
#     step:   d <- MAX(MaxNeg, expr) = expr  (reset + absorb first element)
# mirroring the PageIdx 3-uop machine's FSM wiring. HW-validated exact.
# ---------------------------------------------------------------------------
import dataclasses as _dc
import concourse.dve_spec as _DS
from concourse.dve_uop import N_LANES as _N_LANES, N_STAGES as _N_STAGES
from concourse.dve_uop import DveOpSpec as _DveOpSpec
import concourse.dve_ops as _DO


def _segmaxplus_reference(in0, in1, c0, c1, c2):
    s = np.asarray(in0, np.float32) + np.asarray(in1, np.float32)
    return np.maximum.accumulate(s, axis=-1)


def _segmaxplus_build(ver):
    n_lanes, n_stages = _N_LANES[ver], _N_STAGES[ver]
    spec = _DS.Spec(
        body=_DS.scan(_DS.AluOp.MAX, _DS.Src0 + _DS.Src1),
        reference=_segmaxplus_reference,
    )
    spec2 = _DS._hoist_stream_invariant_ops(spec)
    scans = _DS._collect(spec2.body, _DS.Scan)
    latches = _DS._collect(spec2.body, _DS.Latch)
    placement = _DS._build_placement(spec2, scans, n_stages, n_lanes)
    states = _DS._build_state_machine(spec2, scans, latches, placement)
    assert len(states) == 2  # [seed, steady]
    scan_node = scans[0]
    d = placement.node_stage[scan_node]
    steady2 = _dc.replace(
        states[1],
        trigger=(_DS.Trigger.SRC_TENSOR_DONE, _DS.Trigger.SUB_DIM_DONE,
                 _DS.Trigger.NONE),
        next=(0, 2, 0),
    )
    step = _DS._State(
        placement=placement,
        consume=states[1].consume,
        overrides={d: _DS._Stage(_DS.AluOp.MAX, _DS.MaxNeg, scan_node.expr)},
        trigger=(_DS.Trigger.SRC_TENSOR_DONE, _DS.Trigger.SUB_DIM_DONE,
                 _DS.Trigger.COUNT),
        next=(0, 2, 1),
        repeat=1,
    )
    uops = [_DS._assemble(s) for s in (states[0], steady2, step)]
    for u in uops:
        u.validate(ver)
    return spec, uops


class _SegMaxPlusOp:
    name = "SEG_MAXPLUS_ANT"
    subdim = True

    def __init__(self):
        self.spec, _ = _segmaxplus_build("v3")
        self._compiled = {}

    def compile(self, ver):
        if ver not in self._compiled:
            _, uops = _segmaxplus_build(ver)
            self._compiled[ver] = _DveOpSpec(
                name=self.name,
                opcode=_DO.get_dve_sub_opcode(self.name),
                uops=uops,
                rd1_en=True,
            )
        return self._compiled[ver]


def _register_segmaxplus():
    for o in _DO.OPS:
        if o.name == _SegMaxPlusOp.name:
            return o
    op = _SegMaxPlusOp()
    _DO.OPS.append(op)
    _DO._SUB_OPCODE_FOR_NAME[op.name] = max(_DO._SUB_OPCODE_FOR_NAME.values()) + 1
    _DO.CUSTOM_DVE_SPECS[op.name] = op.spec
    return op


SEG_MAXPLUS = _register_segmaxplus()

B, L, T = 2048, 512, 24
NCORES = 8
SEQ_PER_CORE = B // NCORES  # 256
TILES = 2  # tiles of 128 partitions per core
TP = 128
CHUNK = 32  # time steps per If/DMA chunk
NCH = L // CHUNK  # 8
START_ID, STOP_ID, PAD_ID = 24, 25, 23

F32 = mybir.dt.float32
I32 = mybir.dt.int32
_cache = {}


def _build():
    if "nc" in _cache:
        return _cache["nc"]
    nc = bacc.Bacc("TRN2", target_bir_lowering=False, debug=False, num_devices=NCORES)

    feats_in = nc.dram_tensor("feats", [SEQ_PER_CORE, L, T], F32, kind="ExternalInput")
    init_in = nc.dram_tensor("init_fv", [SEQ_PER_CORE, T], F32, kind="ExternalInput")
    trans_in = nc.dram_tensor("trans_rep", [TP, T * T], F32, kind="ExternalInput")
    nch_in = nc.dram_tensor("nch", [1, TILES], I32, kind="ExternalInput")
    fvh_out = nc.dram_tensor("fvh", [SEQ_PER_CORE, L, T], F32, kind="ExternalOutput")

    with tile.TileContext(nc) as tc:
        with ExitStack() as ctx:
            const_pool = ctx.enter_context(tc.tile_pool(name="const", bufs=1))
            feat_pools = [
                ctx.enter_context(tc.tile_pool(name=f"feat{i}", bufs=1))
                for i in range(TILES)
            ]
            fvh_pools = [
                ctx.enter_context(tc.tile_pool(name=f"fvh{i}", bufs=1))
                for i in range(TILES)
            ]
            cand_pools = [
                ctx.enter_context(tc.tile_pool(name=f"cand{i}", bufs=2))
                for i in range(TILES)
            ]
            trans_sb = const_pool.tile([TP, T * T], F32, name="trans_sb")
            nc.sync.dma_start(trans_sb[:], trans_in.ap())
            trans_3d = trans_sb[:].rearrange("p (g k) -> p g k", g=T)

            nch_sb = const_pool.tile([1, TILES], I32, name="nch_sb")
            nc.sync.dma_start(nch_sb[:], nch_in.ap())
            regs = [
                nc.alloc_registers(
                    f"nch{i}",
                    bass.OrderedSet([mybir.EngineType.DVE, mybir.EngineType.SP]),
                )
                for i in range(TILES)
            ]
            sv = []
            for i in range(TILES):
                nc.regs_load(regs[i], nch_sb[0:1, i : i + 1])
                sv.append(nc.snap(regs[i], donate=False, min_val=1, max_val=NCH))

            # whole-tile feat prefetch (one big DMA per tile)
            feat_sb = []
            for ti in range(TILES):
                s0 = ti * TP
                fsb = feat_pools[ti].tile([TP, L * T], F32, name=f"featsb{ti}")
                nc.sync.dma_start(
                    fsb[:], feats_in[s0 : s0 + TP, :, :].rearrange("p a b -> p (a b)")
                )
                feat_sb.append(fsb)

            # static double-buffered fvh chunk tiles (pool allocation inside
            # If blocks deadlocks the tile scheduler)
            hbufs = [
                [
                    fvh_pools[ti].tile([TP, CHUNK * T], F32, name=f"hb{ti}_{j}")
                    for j in range(2)
                ]
                for ti in range(TILES)
            ]
            prev_fv = [None] * TILES

            def emit_chunk(ti, k):
                s0 = ti * TP
                fvh_ch = hbufs[ti][k % 2]
                if k == 0:
                    nc.sync.dma_start(fvh_ch[:, 0:T], init_in[s0 : s0 + TP, :])
                    prev_fv[ti] = fvh_ch[:, 0:T]
                for lt in range(CHUNK):
                    t = k * CHUNK + lt
                    if t == 0:
                        continue
                    seg = cand_pools[ti].tile(
                        [TP, T * T], F32, tag="c", name=f"seg{ti}"
                    )
                    seg_3d = seg[:].rearrange("p (g k) -> p g k", g=T)
                    fv_b = prev_fv[ti].unsqueeze(1).broadcast_to((TP, T, T))
                    # one fused op: seg[p,n,23] = max_p'(trans[n,p'] + fv[p'])
                    nc.vector._custom_dve(
                        SEG_MAXPLUS, out=seg_3d, in0=trans_3d, in1=fv_b
                    )
                    slot = fvh_ch[:, lt * T : (lt + 1) * T]
                    nc.vector.tensor_add(
                        slot, seg[:, T - 1 :: T], feat_sb[ti][:, t * T : (t + 1) * T]
                    )
                    prev_fv[ti] = slot
                nc.sync.dma_start(
                    fvh_out[s0 : s0 + TP, k * CHUNK : (k + 1) * CHUNK, :],
                    fvh_ch[:].rearrange("p (c t) -> p c t", c=CHUNK),
                )

            # chunk 0 unconditional (both tiles interleave); chunks 1..7 inside
            # nested Ifs, one chain per tile (If contexts must nest strictly)
            for ti in range(TILES):
                emit_chunk(ti, 0)
            for ti in range(TILES):
                with ExitStack() as stk:
                    for k in range(1, NCH):
                        stk.enter_context(tc.If(sv[ti] > k, name=f"t{ti}c{k}"))
                        emit_chunk(ti, k)

    nc.compile()
    _cache["nc"] = nc
    return nc


def _layout(lengths):
    """Device layout permutation: perm[i] = original seq index at device row i.
    Core c rows = [tile_c ; tile_{15-c}] of the length-sorted order."""
    order = np.argsort(lengths, kind="stable")
    tiles = [order[t * TP : (t + 1) * TP] for t in range(2 * NCORES)]
    perm, nch = [], []
    for c in range(NCORES):
        ta, tb = c, 2 * NCORES - 1 - c
        perm.append(tiles[ta])
        perm.append(tiles[tb])
        wa = max(1, -(-int(lengths[tiles[ta]].max()) // CHUNK))
        wb = max(1, -(-int(lengths[tiles[tb]].max()) // CHUNK))
        nch.append((wa, wb))
    return np.concatenate(perm), nch


def run_device(feats, init_fv, trans_rep, perm, nch, trace=False):
    nc = _build()
    feats_d = feats[perm]
    init_d = init_fv[perm]
    in_maps = []
    for c in range(NCORES):
        s0 = c * SEQ_PER_CORE
        in_maps.append(
            {
                "feats": np.ascontiguousarray(feats_d[s0 : s0 + SEQ_PER_CORE]),
                "init_fv": np.ascontiguousarray(init_d[s0 : s0 + SEQ_PER_CORE]),
                "trans_rep": trans_rep,
                "nch": np.array([nch[c]], dtype=np.int32),
            }
        )
    res = run_bass_kernel_spmd(nc, in_maps, list(range(NCORES)), trace=trace)
    fvh = np.concatenate([res.results[c]["fvh"] for c in range(NCORES)], axis=0)
    return fvh, res


def kernel(feats, feats_mask, transitions):
    feats = np.asarray(feats, dtype=np.float32)
    feats_mask = np.asarray(feats_mask)
    transitions = np.asarray(transitions, dtype=np.float32)

    lengths = feats_mask.astype(np.int64).sum(axis=1)  # prefix mask, >= 1
    trans24 = transitions[:T, :T]
    trans_rep = np.ascontiguousarray(
        np.broadcast_to(trans24.reshape(1, T * T), (TP, T * T)), dtype=np.float32
    )
    init_fv = (transitions[:T, START_ID][None, :] + feats[:, 0, :]).astype(np.float32)

    perm, nch = _layout(lengths)
    fvh, _ = run_device(feats, init_fv, trans_rep, perm, nch)
    len_d = lengths[perm]

    # ---- host backtrace in device order (exact f32, matches jax reference) ----
    idx = np.arange(B)
    fv_last = fvh[idx, len_d - 1, :]  # frozen fv at the last valid step
    terminal = fv_last + transitions[STOP_ID, :T][None, :]
    best = np.argmax(terminal, axis=1)
    scores_d = terminal[idx, best].astype(np.float32)

    paths_d = np.full((B, L), PAD_ID, dtype=np.int32)
    cur = best.astype(np.int64)
    for t in range(L - 1, 0, -1):
        valid = t < len_d  # mask[:, t] as bool
        paths_d[valid, t] = cur[valid]
        cand = fvh[:, t - 1, :] + trans24[cur]  # [B, T]
        prev = np.argmax(cand, axis=1)
        cur = np.where(valid, prev, cur)
    paths_d[:, 0] = cur

    paths = np.empty_like(paths_d)
    scores = np.empty_like(scores_d)
    paths[perm] = paths_d
    scores[perm] = scores_d
    return paths.astype(np.int32), scores


# revision 12
# speedup vs baseline: 1.2503x; 1.2051x over previous
"""Viterbi CRF decode kernel for Trainium2 (8 NeuronCores, data-parallel).

Device computes the forward DP (max-plus scan) per sequence and streams the
full fv history back; the host recomputes backpointers only along the taken
path (exact in f32) and does the backtrace.

Layout per core: 256 sequences = 2 tiles x 128 partitions; tags in the free
dim. Per step: cand[s,n,p] = fv[s,p] + trans[n,p] (tensor_tensor with a
broadcast AP), grouped reduce-max over p, then + feat[s,t,n].

Sequences are length-sorted on the host into 16 tiles; core c gets tiles
(c, 15-c) so every core sees ~the same total work. Each tile's time loop is
split into 64-step chunks; chunks past ceil(maxlen/64) are skipped at runtime
via nested tile-If blocks on a register loaded from the per-core "nch" input.
"""
import sys

sys.path.insert(0, "/opt/trn_rl_repo")

import numpy as np
from contextlib import ExitStack

import concourse.bass as bass
import concourse.bacc as bacc
import concourse.tile as tile
import concourse.mybir as mybir
from concourse.bass_utils import run_bass_kernel_spmd

# ---------------------------------------------------------------------------
# Custom DVE op: segmented max-plus (SEG_MAXPLUS_ANT).
# out[p,s,k] = running max over k'<=k of (in0[p,s,k'] + in1[p,s,k']), with the
# running max reset at every page boundary s. The per-page max lands at
# out[p,s,N-1]. Built from Spec(body=scan(MAX, Src0+Src1)) -> [seed, steady],
# plus a hand-constructed SUB_DIM_DONE step state that re-seeds the scan stage
# for exactly the first element of each new page:
#     steady: d <- MAX(CURR_ALU_OUT, expr)   (running max)
#     step:   d <- MAX(MaxNeg, expr) = expr  (reset + absorb first element)
# mirroring the PageIdx 3-uop machine's FSM wiring. HW-validated exact.
# ---------------------------------------------------------------------------
import dataclasses as _dc
import concourse.dve_spec as _DS
from concourse.dve_uop import N_LANES as _N_LANES, N_STAGES as _N_STAGES
from concourse.dve_uop import DveOpSpec as _DveOpSpec
import concourse.dve_ops as _DO


def _segmaxplus_reference(in0, in1, c0, c1, c2):
    s = np.asarray(in0, np.float32) + np.asarray(in1, np.float32)
    return np.maximum.accumulate(s, axis=-1)


def _segmaxplus_build(ver):
    n_lanes, n_stages = _N_LANES[ver], _N_STAGES[ver]
    spec = _DS.Spec(
        body=_DS.scan(_DS.AluOp.MAX, _DS.Src0 + _DS.Src1),
        reference=_segmaxplus_reference,
    )
    spec2 = _DS._hoist_stream_invariant_ops(spec)
    scans = _DS._collect(spec2.body, _DS.Scan)
    latches = _DS._collect(spec2.body, _DS.Latch)
    placement = _DS._build_placement(spec2, scans, n_stages, n_lanes)
    states = _DS._build_state_machine(spec2, scans, latches, placement)
    assert len(states) == 2  # [seed, steady]
    scan_node = scans[0]
    d = placement.node_stage[scan_node]
    steady2 = _dc.replace(
        states[1],
        trigger=(_DS.Trigger.SRC_TENSOR_DONE, _DS.Trigger.SUB_DIM_DONE,
                 _DS.Trigger.NONE),
        next=(0, 2, 0),
    )
    step = _DS._State(
        placement=placement,
        consume=states[1].consume,
        overrides={d: _DS._Stage(_DS.AluOp.MAX, _DS.MaxNeg, scan_node.expr)},
        trigger=(_DS.Trigger.SRC_TENSOR_DONE, _DS.Trigger.SUB_DIM_DONE,
                 _DS.Trigger.COUNT),
        next=(0, 2, 1),
        repeat=1,
    )
    uops = [_DS._assemble(s) for s in (states[0], steady2, step)]
    for u in uops:
        u.validate(ver)
    return spec, uops


class _SegMaxPlusOp:
    name = "SEG_MAXPLUS_ANT"
    subdim = True

    def __init__(self):
        self.spec, _ = _segmaxplus_build("v3")
        self._compiled = {}

    def compile(self, ver):
        if ver not in self._compiled:
            _, uops = _segmaxplus_build(ver)
            self._compiled[ver] = _DveOpSpec(
                name=self.name,
                opcode=_DO.get_dve_sub_opcode(self.name),
                uops=uops,
                rd1_en=True,
            )
        return self._compiled[ver]


def _register_segmaxplus():
    for o in _DO.OPS:
        if o.name == _SegMaxPlusOp.name:
            return o
    op = _SegMaxPlusOp()
    _DO.OPS.append(op)
    _DO._SUB_OPCODE_FOR_NAME[op.name] = max(_DO._SUB_OPCODE_FOR_NAME.values()) + 1
    _DO.CUSTOM_DVE_SPECS[op.name] = op.spec
    return op


SEG_MAXPLUS = _register_segmaxplus()


def _strip_same_engine_waits(nc):
    """Drop DVE-instruction waits on semaphores that only DVE *compute*
    instructions increment. The DVE executes in order and drains its pipe
    between ops, so a same-engine RAW/WAR is already ordered; the semaphore
    wait only adds the completion->sem-post->issue round trip to every
    dependent op. Cross-engine waits (DMA completion etc.) are untouched."""
    fn = nc.m.functions[0]
    updaters = {}
    for b in fn.blocks:
        for inst in b.instructions:
            si = inst.sync_info
            if not si:
                continue
            is_dma = "DMA" in type(inst).__name__ or "dma" in str(
                getattr(inst, "opcode", "")
            ).lower()
            for u in si.on_update:
                key = u.ant_name
                updaters.setdefault(key, set()).add(
                    (inst.engine, is_dma)
                )
    import concourse.mybir as _mb

    dve_only = {
        name
        for name, srcs in updaters.items()
        if all(e == _mb.EngineType.DVE and not d for (e, d) in srcs)
    }
    n_stripped = 0
    for b in fn.blocks:
        for inst in b.instructions:
            if inst.engine != _mb.EngineType.DVE:
                continue
            si = inst.sync_info
            if not si or not si.on_wait:
                continue
            keep = [
                w
                for w in si.on_wait
                if not (
                    w.sync_type == "semaphore"
                    and w.wait_mode == "sem-ge-imm"
                    and w.ant_name in dve_only
                )
            ]
            if len(keep) != len(si.on_wait):
                si.on_wait = keep
                n_stripped += 1
    return n_stripped

B, L, T = 2048, 512, 24
NCORES = 8
SEQ_PER_CORE = B // NCORES  # 256
TILES = 2  # tiles of 128 partitions per core
TP = 128
CHUNK = 32  # time steps per If/DMA chunk
NCH = L // CHUNK  # 8
START_ID, STOP_ID, PAD_ID = 24, 25, 23

F32 = mybir.dt.float32
I32 = mybir.dt.int32
_cache = {}


def _build():
    if "nc" in _cache:
        return _cache["nc"]
    nc = bacc.Bacc("TRN2", target_bir_lowering=False, debug=False, num_devices=NCORES)

    feats_in = nc.dram_tensor("feats", [SEQ_PER_CORE, L, T], F32, kind="ExternalInput")
    init_in = nc.dram_tensor("init_fv", [SEQ_PER_CORE, T], F32, kind="ExternalInput")
    trans_in = nc.dram_tensor("trans_rep", [TP, T * T], F32, kind="ExternalInput")
    nch_in = nc.dram_tensor("nch", [1, TILES], I32, kind="ExternalInput")
    fvh_out = nc.dram_tensor("fvh", [SEQ_PER_CORE, L, T], F32, kind="ExternalOutput")

    with tile.TileContext(nc) as tc:
        with ExitStack() as ctx:
            const_pool = ctx.enter_context(tc.tile_pool(name="const", bufs=1))
            feat_pools = [
                ctx.enter_context(tc.tile_pool(name=f"feat{i}", bufs=1))
                for i in range(TILES)
            ]
            fvh_pools = [
                ctx.enter_context(tc.tile_pool(name=f"fvh{i}", bufs=1))
                for i in range(TILES)
            ]
            cand_pools = [
                ctx.enter_context(tc.tile_pool(name=f"cand{i}", bufs=2))
                for i in range(TILES)
            ]
            trans_sb = const_pool.tile([TP, T * T], F32, name="trans_sb")
            nc.sync.dma_start(trans_sb[:], trans_in.ap())
            trans_3d = trans_sb[:].rearrange("p (g k) -> p g k", g=T)

            nch_sb = const_pool.tile([1, TILES], I32, name="nch_sb")
            nc.sync.dma_start(nch_sb[:], nch_in.ap())
            regs = [
                nc.alloc_registers(
                    f"nch{i}",
                    bass.OrderedSet([mybir.EngineType.DVE, mybir.EngineType.SP]),
                )
                for i in range(TILES)
            ]
            sv = []
            for i in range(TILES):
                nc.regs_load(regs[i], nch_sb[0:1, i : i + 1])
                sv.append(nc.snap(regs[i], donate=False, min_val=1, max_val=NCH))

            # whole-tile feat prefetch (one big DMA per tile)
            feat_sb = []
            for ti in range(TILES):
                s0 = ti * TP
                fsb = feat_pools[ti].tile([TP, L * T], F32, name=f"featsb{ti}")
                nc.sync.dma_start(
                    fsb[:], feats_in[s0 : s0 + TP, :, :].rearrange("p a b -> p (a b)")
                )
                feat_sb.append(fsb)

            # static double-buffered fvh chunk tiles (pool allocation inside
            # If blocks deadlocks the tile scheduler)
            hbufs = [
                [
                    fvh_pools[ti].tile([TP, CHUNK * T], F32, name=f"hb{ti}_{j}")
                    for j in range(2)
                ]
                for ti in range(TILES)
            ]
            for bufs in hbufs:
                for hb in bufs:
                    nc.vector.memset(hb[:], 0.0)
            prev_fv = [None] * TILES

            def emit_chunk(ti, k):
                s0 = ti * TP
                fvh_ch = hbufs[ti][k % 2]
                if k == 0:
                    nc.sync.dma_start(fvh_ch[:, 0:T], init_in[s0 : s0 + TP, :])
                    prev_fv[ti] = fvh_ch[:, 0:T]
                for lt in range(CHUNK):
                    t = k * CHUNK + lt
                    if t == 0:
                        continue
                    seg = cand_pools[ti].tile(
                        [TP, T * T], F32, tag="c", name=f"seg{ti}"
                    )
                    seg_3d = seg[:].rearrange("p (g k) -> p g k", g=T)
                    fv_b = prev_fv[ti].unsqueeze(1).broadcast_to((TP, T, T))
                    # one fused op: seg[p,n,23] = max_p'(trans[n,p'] + fv[p'])
                    nc.vector._custom_dve(
                        SEG_MAXPLUS, out=seg_3d, in0=trans_3d, in1=fv_b
                    )
                    slot = fvh_ch[:, lt * T : (lt + 1) * T]
                    nc.vector.tensor_add(
                        slot, seg[:, T - 1 :: T], feat_sb[ti][:, t * T : (t + 1) * T]
                    )
                    prev_fv[ti] = slot
                nc.sync.dma_start(
                    fvh_out[s0 : s0 + TP, k * CHUNK : (k + 1) * CHUNK, :],
                    fvh_ch[:].rearrange("p (c t) -> p c t", c=CHUNK),
                )

            # chunk 0 unconditional (both tiles interleave); chunks 1..7 inside
            # nested Ifs, one chain per tile (If contexts must nest strictly)
            for ti in range(TILES):
                emit_chunk(ti, 0)
            for ti in range(TILES):
                with ExitStack() as stk:
                    for k in range(1, NCH):
                        stk.enter_context(tc.If(sv[ti] > k, name=f"t{ti}c{k}"))
                        emit_chunk(ti, k)

    _strip_same_engine_waits(nc)
    nc.compile()
    _cache["nc"] = nc
    return nc


def _layout(lengths):
    """Device layout permutation: perm[i] = original seq index at device row i.
    Core c rows = [tile_c ; tile_{15-c}] of the length-sorted order."""
    order = np.argsort(lengths, kind="stable")
    tiles = [order[t * TP : (t + 1) * TP] for t in range(2 * NCORES)]
    perm, nch = [], []
    for c in range(NCORES):
        ta, tb = c, 2 * NCORES - 1 - c
        perm.append(tiles[ta])
        perm.append(tiles[tb])
        wa = max(1, -(-int(lengths[tiles[ta]].max()) // CHUNK))
        wb = max(1, -(-int(lengths[tiles[tb]].max()) // CHUNK))
        nch.append((wa, wb))
    return np.concatenate(perm), nch


def run_device(feats, init_fv, trans_rep, perm, nch, trace=False):
    nc = _build()
    feats_d = feats[perm]
    init_d = init_fv[perm]
    in_maps = []
    for c in range(NCORES):
        s0 = c * SEQ_PER_CORE
        in_maps.append(
            {
                "feats": np.ascontiguousarray(feats_d[s0 : s0 + SEQ_PER_CORE]),
                "init_fv": np.ascontiguousarray(init_d[s0 : s0 + SEQ_PER_CORE]),
                "trans_rep": trans_rep,
                "nch": np.array([nch[c]], dtype=np.int32),
            }
        )
    res = run_bass_kernel_spmd(nc, in_maps, list(range(NCORES)), trace=trace)
    fvh = np.concatenate([res.results[c]["fvh"] for c in range(NCORES)], axis=0)
    return fvh, res


def kernel(feats, feats_mask, transitions):
    feats = np.asarray(feats, dtype=np.float32)
    feats_mask = np.asarray(feats_mask)
    transitions = np.asarray(transitions, dtype=np.float32)

    lengths = feats_mask.astype(np.int64).sum(axis=1)  # prefix mask, >= 1
    trans24 = transitions[:T, :T]
    trans_rep = np.ascontiguousarray(
        np.broadcast_to(trans24.reshape(1, T * T), (TP, T * T)), dtype=np.float32
    )
    init_fv = (transitions[:T, START_ID][None, :] + feats[:, 0, :]).astype(np.float32)

    perm, nch = _layout(lengths)
    fvh, _ = run_device(feats, init_fv, trans_rep, perm, nch)
    len_d = lengths[perm]

    # ---- host backtrace in device order (exact f32, matches jax reference) ----
    idx = np.arange(B)
    fv_last = fvh[idx, len_d - 1, :]  # frozen fv at the last valid step
    terminal = fv_last + transitions[STOP_ID, :T][None, :]
    best = np.argmax(terminal, axis=1)
    scores_d = terminal[idx, best].astype(np.float32)

    paths_d = np.full((B, L), PAD_ID, dtype=np.int32)
    cur = best.astype(np.int64)
    for t in range(L - 1, 0, -1):
        valid = t < len_d  # mask[:, t] as bool
        paths_d[valid, t] = cur[valid]
        cand = fvh[:, t - 1, :] + trans24[cur]  # [B, T]
        prev = np.argmax(cand, axis=1)
        cur = np.where(valid, prev, cur)
    paths_d[:, 0] = cur

    paths = np.empty_like(paths_d)
    scores = np.empty_like(scores_d)
    paths[perm] = paths_d
    scores[perm] = scores_d
    return paths.astype(np.int32), scores


# revision 13
# speedup vs baseline: 1.2528x; 1.0020x over previous
"""Viterbi CRF decode kernel for Trainium2 (8 NeuronCores, data-parallel).

Device computes the forward DP (max-plus scan) per sequence and streams the
full fv history back; the host recomputes backpointers only along the taken
path (exact in f32) and does the backtrace.

Layout per core: 256 sequences = 2 tiles x 128 partitions; tags in the free
dim. Per step: cand[s,n,p] = fv[s,p] + trans[n,p] (tensor_tensor with a
broadcast AP), grouped reduce-max over p, then + feat[s,t,n].

Sequences are length-sorted on the host into 16 tiles; core c gets tiles
(c, 15-c) so every core sees ~the same total work. Each tile's time loop is
split into 64-step chunks; chunks past ceil(maxlen/64) are skipped at runtime
via nested tile-If blocks on a register loaded from the per-core "nch" input.
"""
import sys

sys.path.insert(0, "/opt/trn_rl_repo")

import numpy as np
from contextlib import ExitStack

import concourse.bass as bass
import concourse.bacc as bacc
import concourse.tile as tile
import concourse.mybir as mybir
from concourse.bass_utils import run_bass_kernel_spmd

# ---------------------------------------------------------------------------
# Custom DVE op: segmented max-plus (SEG_MAXPLUS_ANT).
# out[p,s,k] = running max over k'<=k of (in0[p,s,k'] + in1[p,s,k']), with the
# running max reset at every page boundary s. The per-page max lands at
# out[p,s,N-1]. Built from Spec(body=scan(MAX, Src0+Src1)) -> [seed, steady],
# plus a hand-constructed SUB_DIM_DONE step state that re-seeds the scan stage
# for exactly the first element of each new page:
#     steady: d <- MAX(CURR_ALU_OUT, expr)   (running max)
#     step:   d <- MAX(MaxNeg, expr) = expr  (reset + absorb first element)
# mirroring the PageIdx 3-uop machine's FSM wiring. HW-validated exact.
# ---------------------------------------------------------------------------
import dataclasses as _dc
import concourse.dve_spec as _DS
from concourse.dve_uop import N_LANES as _N_LANES, N_STAGES as _N_STAGES
from concourse.dve_uop import DveOpSpec as _DveOpSpec
import concourse.dve_ops as _DO


def _segmaxplus_reference(in0, in1, c0, c1, c2):
    s = np.asarray(in0, np.float32) + np.asarray(in1, np.float32)
    return np.maximum.accumulate(s, axis=-1)


def _segmaxplus_build(ver):
    n_lanes, n_stages = _N_LANES[ver], _N_STAGES[ver]
    spec = _DS.Spec(
        body=_DS.scan(_DS.AluOp.MAX, _DS.Src0 + _DS.Src1),
        reference=_segmaxplus_reference,
    )
    spec2 = _DS._hoist_stream_invariant_ops(spec)
    scans = _DS._collect(spec2.body, _DS.Scan)
    latches = _DS._collect(spec2.body, _DS.Latch)
    placement = _DS._build_placement(spec2, scans, n_stages, n_lanes)
    states = _DS._build_state_machine(spec2, scans, latches, placement)
    assert len(states) == 2  # [seed, steady]
    scan_node = scans[0]
    d = placement.node_stage[scan_node]
    steady2 = _dc.replace(
        states[1],
        trigger=(_DS.Trigger.SRC_TENSOR_DONE, _DS.Trigger.SUB_DIM_DONE,
                 _DS.Trigger.NONE),
        next=(0, 2, 0),
    )
    step = _DS._State(
        placement=placement,
        consume=states[1].consume,
        overrides={d: _DS._Stage(_DS.AluOp.MAX, _DS.MaxNeg, scan_node.expr)},
        trigger=(_DS.Trigger.SRC_TENSOR_DONE, _DS.Trigger.SUB_DIM_DONE,
                 _DS.Trigger.COUNT),
        next=(0, 2, 1),
        repeat=1,
    )
    uops = [_DS._assemble(s) for s in (states[0], steady2, step)]
    for u in uops:
        u.validate(ver)
    return spec, uops


class _SegMaxPlusOp:
    name = "SEG_MAXPLUS_ANT"
    subdim = True

    def __init__(self):
        self.spec, _ = _segmaxplus_build("v3")
        self._compiled = {}

    def compile(self, ver):
        if ver not in self._compiled:
            _, uops = _segmaxplus_build(ver)
            self._compiled[ver] = _DveOpSpec(
                name=self.name,
                opcode=_DO.get_dve_sub_opcode(self.name),
                uops=uops,
                rd1_en=True,
            )
        return self._compiled[ver]


def _register_segmaxplus():
    for o in _DO.OPS:
        if o.name == _SegMaxPlusOp.name:
            return o
    op = _SegMaxPlusOp()
    _DO.OPS.append(op)
    _DO._SUB_OPCODE_FOR_NAME[op.name] = max(_DO._SUB_OPCODE_FOR_NAME.values()) + 1
    _DO.CUSTOM_DVE_SPECS[op.name] = op.spec
    return op


SEG_MAXPLUS = _register_segmaxplus()


def _strip_same_engine_waits(nc):
    """Drop DVE-instruction waits on semaphores that only DVE *compute*
    instructions increment. The DVE executes in order and drains its pipe
    between ops, so a same-engine RAW/WAR is already ordered; the semaphore
    wait only adds the completion->sem-post->issue round trip to every
    dependent op. Cross-engine waits (DMA completion etc.) are untouched."""
    fn = nc.m.functions[0]
    updaters = {}
    for b in fn.blocks:
        for inst in b.instructions:
            si = inst.sync_info
            if not si:
                continue
            is_dma = "DMA" in type(inst).__name__ or "dma" in str(
                getattr(inst, "opcode", "")
            ).lower()
            for u in si.on_update:
                key = u.ant_name
                updaters.setdefault(key, set()).add(
                    (inst.engine, is_dma)
                )
    import concourse.mybir as _mb

    dve_only = {
        name
        for name, srcs in updaters.items()
        if all(e == _mb.EngineType.DVE and not d for (e, d) in srcs)
    }
    n_stripped = 0
    for b in fn.blocks:
        for inst in b.instructions:
            if inst.engine != _mb.EngineType.DVE:
                continue
            si = inst.sync_info
            if not si or not si.on_wait:
                continue
            keep = [
                w
                for w in si.on_wait
                if not (
                    w.sync_type == "semaphore"
                    and w.wait_mode == "sem-ge-imm"
                    and w.ant_name in dve_only
                )
            ]
            if len(keep) != len(si.on_wait):
                si.on_wait = keep
                n_stripped += 1
    return n_stripped

B, L, T = 2048, 512, 24
NCORES = 8
SEQ_PER_CORE = B // NCORES  # 256
TILES = 2  # tiles of 128 partitions per core
TP = 128
CHUNK = 16  # time steps per If/DMA chunk
NCH = L // CHUNK  # 8
START_ID, STOP_ID, PAD_ID = 24, 25, 23

F32 = mybir.dt.float32
I32 = mybir.dt.int32
_cache = {}


def _build():
    if "nc" in _cache:
        return _cache["nc"]
    nc = bacc.Bacc("TRN2", target_bir_lowering=False, debug=False, num_devices=NCORES)

    feats_in = nc.dram_tensor("feats", [SEQ_PER_CORE, L, T], F32, kind="ExternalInput")
    init_in = nc.dram_tensor("init_fv", [SEQ_PER_CORE, T], F32, kind="ExternalInput")
    trans_in = nc.dram_tensor("trans_rep", [TP, T * T], F32, kind="ExternalInput")
    nch_in = nc.dram_tensor("nch", [1, TILES], I32, kind="ExternalInput")
    fvh_out = nc.dram_tensor("fvh", [SEQ_PER_CORE, L, T], F32, kind="ExternalOutput")

    with tile.TileContext(nc) as tc:
        with ExitStack() as ctx:
            const_pool = ctx.enter_context(tc.tile_pool(name="const", bufs=1))
            feat_pools = [
                ctx.enter_context(tc.tile_pool(name=f"feat{i}", bufs=1))
                for i in range(TILES)
            ]
            fvh_pools = [
                ctx.enter_context(tc.tile_pool(name=f"fvh{i}", bufs=1))
                for i in range(TILES)
            ]
            cand_pools = [
                ctx.enter_context(tc.tile_pool(name=f"cand{i}", bufs=2))
                for i in range(TILES)
            ]
            trans_sb = const_pool.tile([TP, T * T], F32, name="trans_sb")
            nc.sync.dma_start(trans_sb[:], trans_in.ap())
            trans_3d = trans_sb[:].rearrange("p (g k) -> p g k", g=T)

            nch_sb = const_pool.tile([1, TILES], I32, name="nch_sb")
            nc.sync.dma_start(nch_sb[:], nch_in.ap())
            regs = [
                nc.alloc_registers(
                    f"nch{i}",
                    bass.OrderedSet([mybir.EngineType.DVE, mybir.EngineType.SP]),
                )
                for i in range(TILES)
            ]
            sv = []
            for i in range(TILES):
                nc.regs_load(regs[i], nch_sb[0:1, i : i + 1])
                sv.append(nc.snap(regs[i], donate=False, min_val=1, max_val=NCH))

            # whole-tile feat prefetch (one big DMA per tile)
            feat_sb = []
            for ti in range(TILES):
                s0 = ti * TP
                fsb = feat_pools[ti].tile([TP, L * T], F32, name=f"featsb{ti}")
                nc.sync.dma_start(
                    fsb[:], feats_in[s0 : s0 + TP, :, :].rearrange("p a b -> p (a b)")
                )
                feat_sb.append(fsb)

            # static double-buffered fvh chunk tiles (pool allocation inside
            # If blocks deadlocks the tile scheduler)
            hbufs = [
                [
                    fvh_pools[ti].tile([TP, CHUNK * T], F32, name=f"hb{ti}_{j}")
                    for j in range(2)
                ]
                for ti in range(TILES)
            ]
            for bufs in hbufs:
                for hb in bufs:
                    nc.vector.memset(hb[:], 0.0)
            prev_fv = [None] * TILES

            def emit_chunk(ti, k):
                s0 = ti * TP
                fvh_ch = hbufs[ti][k % 2]
                if k == 0:
                    nc.sync.dma_start(fvh_ch[:, 0:T], init_in[s0 : s0 + TP, :])
                    prev_fv[ti] = fvh_ch[:, 0:T]
                for lt in range(CHUNK):
                    t = k * CHUNK + lt
                    if t == 0:
                        continue
                    seg = cand_pools[ti].tile(
                        [TP, T * T], F32, tag="c", name=f"seg{ti}"
                    )
                    seg_3d = seg[:].rearrange("p (g k) -> p g k", g=T)
                    fv_b = prev_fv[ti].unsqueeze(1).broadcast_to((TP, T, T))
                    # one fused op: seg[p,n,23] = max_p'(trans[n,p'] + fv[p'])
                    nc.vector._custom_dve(
                        SEG_MAXPLUS, out=seg_3d, in0=trans_3d, in1=fv_b
                    )
                    slot = fvh_ch[:, lt * T : (lt + 1) * T]
                    nc.vector.tensor_add(
                        slot, seg[:, T - 1 :: T], feat_sb[ti][:, t * T : (t + 1) * T]
                    )
                    prev_fv[ti] = slot
                nc.sync.dma_start(
                    fvh_out[s0 : s0 + TP, k * CHUNK : (k + 1) * CHUNK, :],
                    fvh_ch[:].rearrange("p (c t) -> p c t", c=CHUNK),
                )

            # chunk 0 unconditional (both tiles interleave); chunks 1..7 inside
            # nested Ifs, one chain per tile (If contexts must nest strictly)
            for ti in range(TILES):
                emit_chunk(ti, 0)
            for ti in range(TILES):
                with ExitStack() as stk:
                    for k in range(1, NCH):
                        stk.enter_context(tc.If(sv[ti] > k, name=f"t{ti}c{k}"))
                        emit_chunk(ti, k)

    _strip_same_engine_waits(nc)
    nc.compile()
    _cache["nc"] = nc
    return nc


def _layout(lengths):
    """Device layout permutation: perm[i] = original seq index at device row i.
    Core c rows = [tile_c ; tile_{15-c}] of the length-sorted order."""
    order = np.argsort(lengths, kind="stable")
    tiles = [order[t * TP : (t + 1) * TP] for t in range(2 * NCORES)]
    perm, nch = [], []
    for c in range(NCORES):
        ta, tb = c, 2 * NCORES - 1 - c
        perm.append(tiles[ta])
        perm.append(tiles[tb])
        wa = max(1, -(-int(lengths[tiles[ta]].max()) // CHUNK))
        wb = max(1, -(-int(lengths[tiles[tb]].max()) // CHUNK))
        nch.append((wa, wb))
    return np.concatenate(perm), nch


def run_device(feats, init_fv, trans_rep, perm, nch, trace=False):
    nc = _build()
    feats_d = feats[perm]
    init_d = init_fv[perm]
    in_maps = []
    for c in range(NCORES):
        s0 = c * SEQ_PER_CORE
        in_maps.append(
            {
                "feats": np.ascontiguousarray(feats_d[s0 : s0 + SEQ_PER_CORE]),
                "init_fv": np.ascontiguousarray(init_d[s0 : s0 + SEQ_PER_CORE]),
                "trans_rep": trans_rep,
                "nch": np.array([nch[c]], dtype=np.int32),
            }
        )
    res = run_bass_kernel_spmd(nc, in_maps, list(range(NCORES)), trace=trace)
    fvh = np.concatenate([res.results[c]["fvh"] for c in range(NCORES)], axis=0)
    return fvh, res


def kernel(feats, feats_mask, transitions):
    feats = np.asarray(feats, dtype=np.float32)
    feats_mask = np.asarray(feats_mask)
    transitions = np.asarray(transitions, dtype=np.float32)

    lengths = feats_mask.astype(np.int64).sum(axis=1)  # prefix mask, >= 1
    trans24 = transitions[:T, :T]
    trans_rep = np.ascontiguousarray(
        np.broadcast_to(trans24.reshape(1, T * T), (TP, T * T)), dtype=np.float32
    )
    init_fv = (transitions[:T, START_ID][None, :] + feats[:, 0, :]).astype(np.float32)

    perm, nch = _layout(lengths)
    fvh, _ = run_device(feats, init_fv, trans_rep, perm, nch)
    len_d = lengths[perm]

    # ---- host backtrace in device order (exact f32, matches jax reference) ----
    idx = np.arange(B)
    fv_last = fvh[idx, len_d - 1, :]  # frozen fv at the last valid step
    terminal = fv_last + transitions[STOP_ID, :T][None, :]
    best = np.argmax(terminal, axis=1)
    scores_d = terminal[idx, best].astype(np.float32)

    paths_d = np.full((B, L), PAD_ID, dtype=np.int32)
    cur = best.astype(np.int64)
    for t in range(L - 1, 0, -1):
        valid = t < len_d  # mask[:, t] as bool
        paths_d[valid, t] = cur[valid]
        cand = fvh[:, t - 1, :] + trans24[cur]  # [B, T]
        prev = np.argmax(cand, axis=1)
        cur = np.where(valid, prev, cur)
    paths_d[:, 0] = cur

    paths = np.empty_like(paths_d)
    scores = np.empty_like(scores_d)
    paths[perm] = paths_d
    scores[perm] = scores_d
    return paths.astype(np.int32), scores
